# revision 25
# baseline (speedup 1.0000x reference)
"""Trainium2 Bass kernel for a basic RNN layer.

Reference: h_t = relu(concat([x_t, h_{t-1}]) @ W + b), outputs all h_t.
Shapes: x [64, 512, 1024], W [2048, 1024], b [1024]; out [64, 512, 1024] f32.

Strategy (v2: warm-clock pipelined steady state)
------------------------------------------------
Data-parallel over batch: 8 cores x 8 batch rows each.  W splits into
W_x = W[:1024] (precompute u_t = x_t @ W_x + b) and W_h = W[1024:]
(the serial recurrence h_t = relu(u_t + h_{t-1} @ W_h)).

Everything is hidden-major: hidden on SBUF partitions (8 chunks of
128), (time, batch) on the free dim.  Per step the PE runs 64
LDWEIGHTS+MATMUL pairs (8 h_out-chunks x 8 h_in-chunks, W_h tile
stationary, h.T [128, 8] moving) plus 4 identity-weight injections of
u_t into the 4 PSUM quarter banks.

Key measured facts driving this version (from NTFF traces of v1):
  * The PE HAM clock gate kept the whole recurrence at K=4/8
    (1.2 GHz): every step had a ~250 ns relu-wait gap, so no 3.4 us
    window was ever fully busy and the PE never un-throttled.
  * The scheduler-chosen order made each step's relus fire in a burst
    at the end of the step, putting one relu-latency stall on every
    step boundary.

Fixes:
  * W_h is stored as fp8 e3m4 scaled by 64 (exactly representable
    power of two; rel-err vs fp32 reference simulated at 8.2e-3, well
    under the 2e-2 gate).  FWL reads 4 fp8/32-bit vs 2 bf16, halving
    the dominant LDWEIGHTS cost.  W_x and b are pre-scaled by 64 so
    PSUM holds 64*z; the relu epilogue is a fused max(psum,0) * 2^-6
    on the DVE.
  * The PE instruction order is pinned with explicit ordering edges
    (add_dep_helper, sync=False): per step [4 ids | 3 precompute
    filler matmuls | k=0..5 consumers (k-major, m-inner) | k=6,7 per
    quarter, each quarter followed by its relu].  Consumers are
    ordered so each one starts after the relu that produces its h
    chunk; the filler matmuls at the step head cover the last-relu
    latency, so the PE never idles and HAM stays at 2.4 GHz.
  * The u precompute (8 m x 8 k matmuls per 256-column chunk) IS the
    filler stream: it moves off the 135 us serial prologue and into
    the recurrence for free.  Once exhausted, cheap dummy matmuls into
    a scratch PSUM bank keep the clock warm.

All matmul operands are bf16/fp8 with fp32 PSUM accumulation.
"""

import numpy as np
import ml_dtypes

import concourse.bass as bass
import concourse.bacc as bacc
import concourse.tile as tile
import concourse.mybir as mybir
from concourse.bass_utils import run_bass_kernel_spmd
from concourse.tile_rust import add_dep_helper

BF16 = ml_dtypes.bfloat16
F8E3 = ml_dtypes.float8_e3m4

B, T, D, H = 64, 512, 1024, 1024
NCORES = 8
BC = B // NCORES        # batch rows per core = 8
KD = D // 128           # input-dim chunks = 8
KH = H // 128           # hidden-dim chunks = 8
MCH = H // 128          # output-hidden chunks = 8
SW = MCH * BC           # step width in free-dim columns = 64
NT = 512                # precompute moving-chunk columns
TPC = NT // BC          # timesteps covered per chunk = 32
PRE = 2                 # u chunks computed up front
FPS = 1                 # filler pairs per step
WH_SCALE = 64.0


def build_nc(t_steps: int = T):
    """Build the per-core Bass program (SPMD: all cores run this NEFF)."""
    nb = t_steps * BC
    nchunk = nb // NT
    assert nb % NT == 0

    f32 = mybir.dt.float32
    bf16 = mybir.dt.bfloat16
    f8 = mybir.dt.float8e3

    nc = bacc.Bacc("TRN2", target_bir_lowering=False, debug=False)
    xT = nc.dram_tensor("xT", [128, KD * nb], bf16, kind="ExternalInput").ap()
    Wx = nc.dram_tensor("Wx", [128, KD * H], bf16, kind="ExternalInput").ap()
    Wh = nc.dram_tensor("Wh", [128, KH * H], f8, kind="ExternalInput").ap()
    bias = nc.dram_tensor("bias", [128, MCH], f32, kind="ExternalInput").ap()
    ident = nc.dram_tensor("ident", [128, 128], f8, kind="ExternalInput").ap()
    Y = nc.dram_tensor("Y", [t_steps, 128, SW], bf16, kind="ExternalOutput").ap()

    with tile.TileContext(nc) as tc, \
            tc.tile_pool(name="const", bufs=1) as cpool, \
            tc.tile_pool(name="xin", bufs=3) as xpool, \
            tc.tile_pool(name="u", bufs=1) as upool, \
            tc.tile_pool(name="h", bufs=6) as hpool:

        wx_sb = cpool.tile([128, KD * H], bf16, tag="wx")
        wh_sb = cpool.tile([128, KH * H], f8, tag="wh")
        b_sb = cpool.tile([128, MCH], f32, tag="bias")
        id_sb = cpool.tile([128, 128], f8, tag="ident")
        u_sb = upool.tile([128, t_steps * SW], bf16)
        # u column layout: t*SW + m*BC + b, matching the recurrence psum.
        uv = u_sb[:].rearrange("p (t m b) -> p t m b", m=MCH, b=BC)

        # Explicit per-engine ordering chains: the Tile scheduler's own
        # order produced end-of-step relu bursts; pin the exact order.
        state = {"pe": None, "dve": None}

        def chain(inst, key):
            prev = state[key]
            if prev is not None:
                add_dep_helper(inst.ins, prev.ins, sync=False, reason=f"{key}-order")
            state[key] = inst
            return inst

        def pe(inst):
            return chain(inst, "pe")

        def dve(inst):
            return chain(inst, "dve")

        # ---- constant loads; wh/id after the first x chunks so the
        # precompute's critical DMAs go first on the queue ----
        for k in range(KD):
            nc.sync.dma_start(wx_sb[:, k * H:(k + 1) * H], Wx[:, k * H:(k + 1) * H])
        nc.sync.dma_start(b_sb[:], bias[:])

        def emit_x_dma(n):
            xn = xpool.tile([128, KD * NT], bf16, tag="xn")
            for k in range(KD):
                nc.sync.dma_start(
                    xn[:, k * NT:(k + 1) * NT],
                    xT[:, k * nb + n * NT: k * nb + (n + 1) * NT],
                )
            return xn

        class Pre:
            """U-precompute pair emitter: one (m, k) matmul per call,
            with auto x-DMA prefetch one chunk ahead and the psum+bias
            -> bf16 epilogue on group completion."""

            def __init__(self):
                self.tasks = [(n, m) for n in range(nchunk) for m in range(MCH)]
                self.ti = 0
                self.k = 0
                self.ps = None
                self.xn = {}

            def emit_pair(self) -> bool:
                if self.ti >= len(self.tasks):
                    return False
                n, m = self.tasks[self.ti]
                if n not in self.xn:
                    self.xn[n] = emit_x_dma(n)
                if m == 0 and self.k == 0:
                    for np_ in (n + 1, n + 2):
                        if np_ < nchunk and np_ not in self.xn:
                            self.xn[np_] = emit_x_dma(np_)
                if self.k == 0:
                    self.ps = self.pool.tile([128, NT], f32, tag="pu")
                k = self.k
                pe(nc.tensor.matmul(
                    self.ps[:],
                    wx_sb[:, k * H + m * 128: k * H + (m + 1) * 128],
                    self.xn[n][:, k * NT:(k + 1) * NT],
                    start=(k == 0),
                    stop=(k == KD - 1),
                ))
                self.k += 1
                if self.k == KD:
                    dve(nc.vector.tensor_scalar_add(
                        uv[:, n * TPC:(n + 1) * TPC, m, :],
                        self.ps[:],
                        b_sb[:, m:m + 1],
                    ))
                    self.k = 0
                    self.ti += 1
                return True

        pre = Pre()
        pre.xn[0] = emit_x_dma(0)
        nc.sync.dma_start(id_sb[:], ident[:])
        for k in range(KH):
            nc.sync.dma_start(wh_sb[:, k * H:(k + 1) * H], Wh[:, k * H:(k + 1) * H])

        # ---- upfront precompute: ALL chunks, dense N=256 matmuls ----
        # (pu pool scoped to the upfront so the recurrence gets all 8
        # PSUM banks -> 2-step bank-reuse distance, no WAR waits)
        # The HAM activity monitor tracks PE *array* activity: the
        # LDW-dominated recurrence (N=8 moving) can never re-warm the
        # clock, so chasing K=8/8 with in-step fillers just paid 2x for
        # the precompute at 1.2 GHz.  Run the whole precompute upfront
        # (array-dense, runs at 2.4 GHz) and keep the recurrence
        # filler-free; the staggered quarter schedule alone covers the
        # relu latencies.
        pu_pool = None  # opened in _run_recurrence alongside ph

        # ---- recurrence ----
        # Persistent scratch tile for warm-clock dummies: allocating a
        # fresh pu tile per dummy serializes ~300ns on tile-release
        # latency (bufs=1); one tile + repeated start/stop groups has
        # no release traffic.
        _run_recurrence(
            nc, tc, pe, dve, pre, wh_sb, id_sb, u_sb, Y, hpool, t_steps)

    nc.compile()  # bacc passes: wait splitting, reg alloc, nop fusion, ...
    return nc


def _run_recurrence(nc, tc, pe, dve, pre, wh_sb, id_sb, u_sb, Y, hpool, t_steps):
    f32 = mybir.dt.float32
    bf16 = mybir.dt.bfloat16
    with tc.tile_pool(name="ph", bufs=6, space="PSUM") as ph_pool, \
            tc.tile_pool(name="pu", bufs=2, space="PSUM") as pu_pool:
        pre.pool = pu_pool
        # upfront: PRE chunks so the in-loop filler stream (FPS=2, the
        # exact steady-state u production rate) always stays a chunk
        # ahead of consumption
        for _ in range(PRE * MCH * KD):
            pre.emit_pair()
        scratch = {"ps": None}

        def emit_dummy():
            if scratch["ps"] is None:
                scratch["ps"] = pu_pool.tile([128, NT], f32, tag="pu", name="scratch")
            pe(nc.tensor.matmul(
                scratch["ps"][:, :384], wh_sb[:, :128], u_sb[:, :384],
                start=True, stop=True,
            ))

        h_prev = None
        for t in range(t_steps):
            h_new = hpool.tile([128, SW], bf16, tag="h")
            qps = []
            for q in range(4):
                ps = ph_pool.tile([128, 2 * BC], f32, tag="ph")
                qps.append(ps)
                pe(nc.tensor.matmul(
                    ps[:],
                    id_sb[:],
                    u_sb[:, t * SW + 2 * q * BC: t * SW + 2 * (q + 1) * BC],
                    start=True,
                    stop=(t == 0),
                ))
            # Dense filler bridge over the pipeline-fill steps: the
            # first steps have little PE work while the relu chain
            # spins up; a single long idle there re-throttles the PE
            # clock (HAM MID window) and it never recovers.
            n_fill = {0: 7, 1: 5, 2: 4, 3: 3, 4: 2, 5: 2}.get(t, FPS)
            for _ in range(n_fill):
                if not pre.emit_pair():
                    emit_dummy()
            if t > 0:
                # small pad: the head cover (ids + filler) runs ~235ns
                # short of relu_A(t-1)'s semaphore visibility
                if scratch["ps"] is None:
                    scratch["ps"] = pu_pool.tile(
                        [128, NT], f32, tag="pu", name="scratch")
                pe(nc.tensor.matmul(
                    scratch["ps"][:, :128], wh_sb[:, :128], u_sb[:, :128],
                    start=True, stop=True,
                ))
            if t > 0:
                # Staggered quarter completion: finish A and B's
                # accumulation groups mid-step (their k=6,7 deferred
                # just far enough to clear last step's relu_D), so
                # their relus fire early; C and D follow.  This keeps
                # every relu well ahead of its consumers AND ahead of
                # the psum-bank reuse by the next step's id-injects.
                def kq(q, ks):
                    for k in ks:
                        for mq in range(2):
                            m = 2 * q + mq
                            pe(nc.tensor.matmul(
                                qps[q][:, mq * BC:(mq + 1) * BC],
                                wh_sb[:, k * H + m * 128: k * H + (m + 1) * 128],
                                h_prev[:, k * BC:(k + 1) * BC],
                                start=False,
                                stop=(k == KH - 1 and mq == 1),
                            ))

                def relu(q):
                    dve(nc.vector.tensor_scalar(
                        h_new[:, 2 * q * BC: 2 * (q + 1) * BC],
                        qps[q][:],
                        0.0,
                        1.0 / WH_SCALE,
                        mybir.AluOpType.max,
                        mybir.AluOpType.mult,
                    ))

                lo, hi = range(KH - 2), (KH - 2, KH - 1)
                kq(0, lo)
                kq(1, lo)
                kq(0, hi)
                relu(0)
                kq(1, hi)
                relu(1)
                kq(2, lo)
                kq(2, hi)
                relu(2)
                kq(3, lo)
                kq(3, hi)
                relu(3)
            else:
                for q in range(4):
                    dve(nc.vector.tensor_scalar(
                        h_new[:, 2 * q * BC: 2 * (q + 1) * BC],
                        qps[q][:],
                        0.0,
                        1.0 / WH_SCALE,
                        mybir.AluOpType.max,
                        mybir.AluOpType.mult,
                    ))
            nc.sync.dma_start(Y[t], h_new[:])
            h_prev = h_new


def _prep_inputs(x: np.ndarray, W: np.ndarray, b: np.ndarray, t_steps: int):
    """Host-side reshapes/casts into the per-core hidden-major layout."""
    nb = t_steps * BC
    Wx, Wh = W[:D], W[D:]
    # [d, h] -> [128, kd*H] with partition = d % 128 (within chunk);
    # W_x and b carry the 64x scale so PSUM holds 64*z and the fused
    # relu multiplies by 2^-6
    wx_np = np.ascontiguousarray(
        (Wx * WH_SCALE).reshape(KD, 128, H).transpose(1, 0, 2).reshape(128, KD * H)
    ).astype(BF16)
    wh_np = np.ascontiguousarray(
        np.clip(Wh * WH_SCALE, -15.5, 15.5)
        .reshape(KH, 128, H).transpose(1, 0, 2).reshape(128, KH * H)
    ).astype(F8E3)
    b_np = np.ascontiguousarray((b * WH_SCALE).reshape(MCH, 128).T).astype(np.float32)

    in_maps = []
    for c in range(NCORES):
        xc = x[c * BC:(c + 1) * BC, :t_steps]  # [BC, t, D]
        # xT[p, k*nb + t*BC + b] = xc[b, t, k*128+p]
        xt = (
            xc.transpose(2, 1, 0)              # [D, t, BC]
            .reshape(KD, 128, nb)
            .transpose(1, 0, 2)
            .reshape(128, KD * nb)
        )
        in_maps.append({
            "xT": np.ascontiguousarray(xt).astype(BF16),
            "Wx": wx_np,
            "Wh": wh_np,
            "bias": b_np,
            "ident": np.eye(128, dtype=F8E3),
        })
    return in_maps


def _assemble_output(results, t_steps: int) -> np.ndarray:
    """[t, 128, SW] bf16 per core -> [B, t, H] f32."""
    y = np.empty((B, t_steps, H), dtype=np.float32)
    for c, res in enumerate(results):
        yc = np.asarray(res["Y"]).astype(np.float32)       # [t, 128, SW]
        yc = yc.reshape(t_steps, 128, MCH, BC).transpose(3, 0, 2, 1)
        y[c * BC:(c + 1) * BC] = yc.reshape(BC, t_steps, H)
    return y


def kernel(x: np.ndarray, W: np.ndarray, b: np.ndarray, **run_kwargs) -> np.ndarray:
    t_steps = x.shape[1]
    nc = build_nc(t_steps)
    in_maps = _prep_inputs(np.asarray(x), np.asarray(W), np.asarray(b), t_steps)
    res = run_bass_kernel_spmd(nc, in_maps, core_ids=list(range(NCORES)), **run_kwargs)
    out = _assemble_output(res.results, t_steps)
    if run_kwargs:
        kernel.last_result = res  # stash for profiling harnesses
    return out
